# revision 10
# baseline (speedup 1.0000x reference)
"""Single-head causal cross-attention on 8 Trainium2 NeuronCores.

Problem: B=8, S=2048, D=1024, HS=64 (fp32).
    q = query @ Wq ; k = key @ Wk ; v = value @ Wv        [B, S, HS]
    out = softmax(causal(q k^T / sqrt(HS))) @ v           [B, S, HS]

Sharding: batch across the 8 cores (one batch element per core), weights
replicated. No collectives.

Per-core design. The kernel is DMA-bound: the projections contract over
d=1024, so query/key/value must reach the PE with d on the partition axis,
but they are [s, d] row-major fp32 in HBM and the transposing DMA (xbar)
only moves 2-byte units with contiguous source rows. Every input byte
therefore flows through the xbar (~8.1us per [512, 2048-unit] chunk, ~97us
for all 12 chunks) and that stream IS the critical path. Everything else is
arranged to hide inside it:

* Transposed loads land unit 128g+p of row s at xt[p, g, s]; odd partitions
  carry the fp32's high half-words (truncated bf16), even partitions carry
  the low half-words (garbage). Garbage partitions are sanitized with a
  per-partition uint16 min-mask on DVE (integer-valued compare, NaN-proof),
  and the matching weight rows are zero so they contribute exactly 0.
* Weights skip the DRAM staging roundtrip: W is loaded dense [64, 16, 64]
  (partition j = d%64), then permutation matmuls scatter rows to odd
  partitions -- PSUM start=True zero-fills the even partitions exactly.
  Scaled by (1 + 2^-9) (compensates the input truncation's toward-zero
  bias) and rounded to bf16 in the PSUM->SBUF copy.
* STREAMING SCHEDULE: load q0..q3 first, then (k_c, v_c) pairs. After each
  kv pair, the score/PV blocks (qc, c) for ALL qc >= c run immediately,
  accumulating into four concurrent PSUM tiles u[qc] (softmax numerator
  and denominator together); u[qc] completes at c == qc and is normalized
  and stored right away. Only the last 4-tile block plus one normalize
  remains after the final DMA, and attention PE work spreads across the
  whole load phase instead of tailing it.
* The PE queue is headed (after the tiny weight matmuls) by the projection
  of q3 -- the LAST q chunk -- so the PE's unavoidable idle time happens
  once, up front, and the subsequent matmul stream runs dense enough for
  the tensor engine to ramp to its top p-state and stay there.
* Scores are computed TRANSPOSED (scoresT[k, q] = kT.T @ qT, fp32r at full
  PE rate) so softmax's reduction runs along the PE contraction axis: exp on
  ACT (1/sqrt(HS) scale fused, no max-subtraction needed -- |scores| <~ 8 by
  construction), then one PV accumulation group with v_ext = [v | 1].
* The small [65, QC] results are PE-transposed back, rows normalized by the
  reciprocal of column 64 on DVE, and stored batched on the ACT HWDGE ring
  (SP stays dedicated to the input transposes).
"""

import sys

for _p in ("/opt/trn_rl_repo",):
    if _p not in sys.path:
        sys.path.insert(0, _p)

import numpy as np

import concourse.bass as bass
import concourse.mybir as mybir
import concourse.tile as tile
from concourse import bacc
from concourse.masks import make_identity

B, S, D, HS = 8, 2048, 1024, 64
N_CORES = 8
QC = 512            # q/s chunk (matmul moving free dim)
KT = 128            # k-tile
NG = 2 * D // 128   # 16 unit-groups of 128 units (64 d-values) each
N_QC = S // QC      # 4
N_KT = S // KT      # 16
W_COMP = 1.0 + 2.0 ** -9   # compensates bf16-truncation bias of the inputs

F32 = mybir.dt.float32
F32R = mybir.dt.float32r
BF16 = mybir.dt.bfloat16
U16 = mybir.dt.uint16
I32 = mybir.dt.int32


def build_body(tc, out_d, q_d, k_d, v_d, w_d):
    nc = tc.nc
    Exp = mybir.ActivationFunctionType.Exp
    AND = mybir.AluOpType.bitwise_and

    with tc.tile_pool(name="const", bufs=1) as const_pool:
        ident = const_pool.tile([128, 128], F32, tag="ident")
        make_identity(nc, ident[:])
        ones_col = const_pool.tile([128, 1], F32, tag="ones_col")
        nc.gpsimd.memset(ones_col[:], 1.0)

        # odd-partition keep-mask for the unit-interleaved layout:
        # fp32 per-partition scalar, 65535.0 on odd partitions / 0.0 on even.
        pidx = const_pool.tile([128, 1], I32, tag="pidx")
        nc.gpsimd.iota(pidx[:], pattern=[[1, 1]], base=0, channel_multiplier=1)
        podd = const_pool.tile([128, 1], I32, tag="podd")
        nc.vector.tensor_scalar(podd[:], pidx[:], 1, None, op0=AND)
        pmask_i = const_pool.tile([128, 1], I32, tag="pmask_i")
        nc.vector.tensor_scalar(pmask_i[:], podd[:], 0xFFFF, None,
                                op0=mybir.AluOpType.mult)
        andmask = const_pool.tile([128, 1], F32, tag="andmask")
        nc.vector.tensor_copy(andmask[:], pmask_i[:])

        # perm[p, 2p+1] = 1.0: as matmul lhsT it scatters 64 dense rows onto
        # odd partitions of a 128-partition PSUM tile (even rows -> 0.0).
        perm = const_pool.tile([64, 128], F32, tag="perm")
        nc.gpsimd.memset(perm[:], 1.0)
        nc.gpsimd.affine_select(
            out=perm[:],
            in_=perm[:],
            compare_op=mybir.AluOpType.is_equal,
            fill=0.0,
            base=-1,
            pattern=[[1, 128]],
            channel_multiplier=-2,
        )

        # Diagonal-block causal masks: mask[j][k_l, q_l] = 1.0 iff
        # q_l >= k_l + 128*j, else 0.0.
        masks = []
        for j in range(QC // KT):
            m = const_pool.tile([128, QC], F32, tag=f"mask{j}", name=f"mask{j}")
            nc.gpsimd.memset(m[:], 1.0)
            nc.gpsimd.affine_select(
                out=m[:],
                in_=m[:],
                compare_op=mybir.AluOpType.is_ge,
                fill=0.0,
                base=-(KT * j),
                pattern=[[1, QC]],
                channel_multiplier=-1,
            )
            masks.append(m)

        with (
            tc.tile_pool(name="xt", bufs=6) as xt_pool,
            tc.tile_pool(name="ksb", bufs=1) as k_sb_pool,
            tc.tile_pool(name="qsb", bufs=1) as q_sb_pool,
            tc.tile_pool(name="vsb", bufs=2) as v_sb_pool,
            tc.tile_pool(name="vext", bufs=1) as vext_pool,
            tc.tile_pool(name="wtmp", bufs=2) as wtmp_pool,
            tc.tile_pool(name="pacc", bufs=2, space="PSUM") as psum_acc,
            tc.tile_pool(name="pscore", bufs=2, space="PSUM") as psum_s,
            tc.tile_pool(name="pu", bufs=1, space="PSUM") as psum_u,
            tc.tile_pool(name="expp", bufs=4) as exp_pool,
            tc.tile_pool(name="usb", bufs=2) as usb_pool,
            tc.tile_pool(name="outsb", bufs=2) as out_pool,
            tc.tile_pool(name="recip", bufs=4) as recip_pool,
        ):
            # ---- input DMAs: q0..q3 first, then (k_c, v_c) pairs. Issue
            # the q transposes before anything else so the SP ring starts
            # the xbar stream at t~0.
            dma_count = [0]

            def issue_dma(xd, c):
                xbf = xd.ap().bitcast(BF16)  # [S, 2D] units
                xt = xt_pool.tile([128, NG, QC], BF16, tag="xt", name="xt")
                # Alternate the two HWDGE rings so one ring's descriptor
                # drain overlaps the other ring's issue.
                eng = nc.sync if dma_count[0] % 2 == 0 else nc.scalar
                dma_count[0] += 1
                eng.dma_start(
                    out=xt[:],
                    in_=xbf[c * QC:(c + 1) * QC, :],
                    transpose=True,
                )
                return xt

            q_xt = [issue_dma(q_d, c) for c in range(N_QC)]

            # ---- weights (ACT ring + a few tiny PE matmuls), overlapped
            # with the q transposes.
            w_all = []
            for wi in range(3):
                wdense = wtmp_pool.tile([64, NG, HS], F32, tag="wd", name="wd")
                nc.gpsimd.dma_start(
                    out=wdense[:],
                    in_=w_d[wi].ap().rearrange("(g j) h -> j g h", j=64),
                )
                wps = psum_s.tile([128, NG * HS // 2], F32, tag="st",
                                  name="wps")
                wps2 = psum_s.tile([128, NG * HS // 2], F32, tag="st",
                                   name="wps2")
                wflat = wdense[:].rearrange("j g h -> j (g h)")
                half = NG * HS // 2
                nc.tensor.matmul(wps[:], lhsT=perm[:], rhs=wflat[:, 0:half])
                nc.tensor.matmul(wps2[:], lhsT=perm[:],
                                 rhs=wflat[:, half:2 * half])
                wa = const_pool.tile([128, NG, HS], BF16, tag=f"w{wi}",
                                     name=f"w{wi}")
                waf = wa[:].rearrange("p g h -> p (g h)")
                nc.scalar.mul(waf[:, 0:half], wps[:], W_COMP)
                nc.scalar.mul(waf[:, half:2 * half], wps2[:], W_COMP)
                w_all.append(wa)

            def project(xt, wi, dst):
                """Sanitize even partitions of xt, project through
                w_all[wi] into dst ([64, QC] slice)."""
                flat = xt[:].rearrange("p g s -> p (g s)").bitcast(U16)
                nc.vector.tensor_scalar(flat, flat, andmask[:], None,
                                        op0=mybir.AluOpType.min)
                acc = psum_acc.tile([HS, QC], F32, tag="acc", name="acc")
                for g in range(NG):
                    nc.tensor.matmul(
                        acc[:],
                        lhsT=w_all[wi][:, g, :],
                        rhs=xt[:, g, :],
                        start=(g == 0),
                        stop=(g == NG - 1),
                    )
                nc.scalar.copy(dst, acc[:])

            # Project q3 FIRST: it depends on the last q transpose, so the
            # PE queue head consolidates the engine's startup idle into one
            # block, after which q0..q2 behind it are data-ready and the
            # matmul stream runs dense (p-state ramps once).
            qT = q_sb_pool.tile([HS, N_QC, QC], F32R, tag="qT", name="qT")
            project(q_xt[3], 0, qT[:, 3, :])
            for qc in range(N_QC - 1):
                project(q_xt[qc], 0, qT[:, qc, :])

            kT = k_sb_pool.tile([HS, S], F32R, tag="kT", name="kT")
            v_ext = [None] * N_KT
            u = [None] * N_QC
            n_done = [0] * N_QC

            for c in range(N_QC):
                k_xt = issue_dma(k_d, c)
                v_xt = issue_dma(v_d, c)
                project(k_xt, 1, kT[:, c * QC:(c + 1) * QC])
                vsb = v_sb_pool.tile([HS, QC], F32, tag="vsb", name="vsb")
                project(v_xt, 2, vsb[:])
                # v_ext[kt] = [v_rows | 1] : [128, HS+1] per k-tile.
                for j in range(QC // KT):
                    kt = c * (QC // KT) + j
                    pt = psum_s.tile([KT, QC], F32, tag="st", name="vtr")
                    nc.tensor.transpose(
                        pt[:, 0:HS],
                        vsb[:, j * KT:(j + 1) * KT],
                        ident[0:HS, 0:HS],
                    )
                    vx = vext_pool.tile([KT, HS + 1], F32R, tag=f"vext{kt}",
                                        name=f"vext{kt}")
                    nc.scalar.copy(vx[:, 0:HS], pt[:, 0:HS])
                    nc.scalar.copy(vx[:, HS:HS + 1], ones_col[:])
                    v_ext[kt] = vx

                # Score/PV blocks (qc, c) for every q chunk that attends to
                # this kv chunk; u[qc] accumulates across c in its own PSUM
                # bank and closes at c == qc.
                for qc in range(c, N_QC):
                    if u[qc] is None:
                        u[qc] = psum_u.tile([HS + 1, QC], F32, tag=f"u{qc}",
                                            name=f"u{qc}")
                    n_kt_total = (qc + 1) * (QC // KT)
                    for j in range(QC // KT):
                        kt = c * (QC // KT) + j
                        st = psum_s.tile([KT, QC], F32, tag="st", name="st")
                        nc.tensor.matmul(
                            st[:],
                            lhsT=kT[:, kt * KT:(kt + 1) * KT],
                            rhs=qT[:, qc, :],
                        )
                        et = exp_pool.tile([KT, QC], F32R, tag="et", name="et")
                        nc.scalar.activation(et[:], st[:], Exp,
                                             scale=float(HS) ** -0.5)
                        if qc == c:  # diagonal block: zero invalid region
                            nc.vector.tensor_mul(et[:], et[:], masks[j][:])
                        nc.tensor.matmul(
                            u[qc][:],
                            lhsT=v_ext[kt][:],
                            rhs=et[:],
                            start=(n_done[qc] == 0),
                            stop=(n_done[qc] == n_kt_total - 1),
                        )
                        n_done[qc] += 1

                # u[c] is complete: transpose back, normalize, store on the
                # ACT ring.
                usb = usb_pool.tile([HS + 1, QC], F32, tag="usb", name="usb")
                nc.scalar.copy(usb[:], u[c][:])
                osb = out_pool.tile([128, (QC // 128) * HS], F32,
                                    tag="osb", name="osb")
                for t in range(QC // 128):
                    po = psum_s.tile([128, QC], F32, tag="st", name="po")
                    nc.tensor.transpose(
                        po[:, 0:HS + 1],
                        usb[:, t * 128:(t + 1) * 128],
                        ident[0:HS + 1, 0:HS + 1],
                    )
                    rc = recip_pool.tile([128, 1], F32, tag="rc", name="rc")
                    nc.vector.reciprocal(rc[:], po[:, HS:HS + 1])
                    nc.vector.tensor_scalar_mul(
                        osb[:, t * HS:(t + 1) * HS], po[:, 0:HS], rc[:]
                    )
                dst = (
                    out_d.ap()[c * QC:(c + 1) * QC, :]
                    .rearrange("(t p) h -> p t h", p=128)
                )
                # SWDGE (Pool ring): keeps output stores off the HWDGE
                # queue semaphores, whose recycling barriers would otherwise
                # make later input transposes wait on earlier stores.
                nc.gpsimd.dma_start(
                    out=dst,
                    in_=osb[:].rearrange("p (t h) -> p t h", t=QC // 128),
                )


_NC_CACHE = {}


def build_nc(debug=False, reps=1):
    key = ("nc", debug, reps)
    if key in _NC_CACHE:
        return _NC_CACHE[key]
    nc = bacc.Bacc(
        "TRN2",
        target_bir_lowering=False,
        debug=debug,
        num_devices=N_CORES,
    )
    q_d = nc.dram_tensor("query", [S, D], F32, kind="ExternalInput")
    k_d = nc.dram_tensor("key", [S, D], F32, kind="ExternalInput")
    v_d = nc.dram_tensor("value", [S, D], F32, kind="ExternalInput")
    wq_d = nc.dram_tensor("Wq", [D, HS], F32, kind="ExternalInput")
    wk_d = nc.dram_tensor("Wk", [D, HS], F32, kind="ExternalInput")
    wv_d = nc.dram_tensor("Wv", [D, HS], F32, kind="ExternalInput")
    out_d = nc.dram_tensor("out", [S, HS], F32, kind="ExternalOutput")

    with tile.TileContext(nc) as tc:
        for _ in range(reps):
            build_body(tc, out_d, q_d, k_d, v_d, [wq_d, wk_d, wv_d])
    nc.compile()
    _NC_CACHE[key] = nc
    return nc


def make_in_maps(query, key, value, Wq, Wk, Wv):
    query = np.ascontiguousarray(query, dtype=np.float32)
    key = np.ascontiguousarray(key, dtype=np.float32)
    value = np.ascontiguousarray(value, dtype=np.float32)
    Wq = np.ascontiguousarray(Wq, dtype=np.float32)
    Wk = np.ascontiguousarray(Wk, dtype=np.float32)
    Wv = np.ascontiguousarray(Wv, dtype=np.float32)
    return [
        {
            "query": query[b],
            "key": key[b],
            "value": value[b],
            "Wq": Wq,
            "Wk": Wk,
            "Wv": Wv,
        }
        for b in range(N_CORES)
    ]


def kernel(query, key, value, Wq, Wk, Wv, trace=False):
    from concourse.bass_utils import run_bass_kernel_spmd

    nc = build_nc()
    in_maps = make_in_maps(query, key, value, Wq, Wk, Wv)
    res = run_bass_kernel_spmd(nc, in_maps, core_ids=list(range(N_CORES)), trace=trace)
    out = np.stack([res.results[b]["out"] for b in range(N_CORES)], axis=0)
    if trace:
        kernel.last_results = res
    return out


# revision 11
# speedup vs baseline: 1.1049x; 1.1049x over previous
"""Single-head causal cross-attention on 8 Trainium2 NeuronCores.

Problem: B=8, S=2048, D=1024, HS=64 (fp32).
    q = query @ Wq ; k = key @ Wk ; v = value @ Wv        [B, S, HS]
    out = softmax(causal(q k^T / sqrt(HS))) @ v           [B, S, HS]

Sharding: batch across the 8 cores (one batch element per core), weights
replicated. No collectives.

Per-core design. The kernel is DMA-bound: the projections contract over
d=1024, so query/key/value must reach the PE with d on the partition axis,
but they are [s, d] row-major fp32 in HBM and the transposing DMA (xbar)
only moves 2-byte units with contiguous source rows. Every input byte
therefore flows through the xbar (~8.1us per [512, 2048-unit] chunk, ~97us
for all 12 chunks) and that stream IS the critical path. Everything else is
arranged to hide inside it:

* Transposed loads land unit 128g+p of row s at xt[p, g, s]; odd partitions
  carry the fp32's high half-words (truncated bf16), even partitions carry
  the low half-words (garbage). Garbage partitions are sanitized with a
  per-partition uint16 min-mask on DVE (integer-valued compare, NaN-proof),
  and the matching weight rows are zero so they contribute exactly 0.
* Weights skip the DRAM staging roundtrip: W is loaded dense [64, 16, 64]
  (partition j = d%64), then permutation matmuls scatter rows to odd
  partitions -- PSUM start=True zero-fills the even partitions exactly.
  Scaled by (1 + 2^-9) (compensates the input truncation's toward-zero
  bias) and rounded to bf16 in the PSUM->SBUF copy.
* STREAMING SCHEDULE: load q0..q3 first, then (k_c, v_c) pairs. After each
  kv pair, the score/PV blocks (qc, c) for ALL qc >= c run immediately,
  accumulating into four concurrent PSUM tiles u[qc] (softmax numerator
  and denominator together); u[qc] completes at c == qc and is normalized
  and stored right away. Only the last 4-tile block plus one normalize
  remains after the final DMA, and attention PE work spreads across the
  whole load phase instead of tailing it.
* The PE queue is headed (after the tiny weight matmuls) by the projection
  of q3 -- the LAST q chunk -- so the PE's unavoidable idle time happens
  once, up front, and the subsequent matmul stream runs dense enough for
  the tensor engine to ramp to its top p-state and stay there.
* Scores are computed TRANSPOSED (scoresT[k, q] = kT.T @ qT, fp32r at full
  PE rate) so softmax's reduction runs along the PE contraction axis: exp on
  ACT (1/sqrt(HS) scale fused, no max-subtraction needed -- |scores| <~ 8 by
  construction), then one PV accumulation group with v_ext = [v | 1].
* The small [65, QC] results are PE-transposed back, rows normalized by the
  reciprocal of column 64 on DVE, and stored batched on the ACT HWDGE ring
  (SP stays dedicated to the input transposes).
"""

import sys

for _p in ("/opt/trn_rl_repo",):
    if _p not in sys.path:
        sys.path.insert(0, _p)

import numpy as np

import concourse.bass as bass
import concourse.mybir as mybir
import concourse.tile as tile
from concourse import bacc
from concourse.masks import make_identity

B, S, D, HS = 8, 2048, 1024, 64
N_CORES = 8
QC = 512            # q/s chunk (matmul moving free dim)
KT = 128            # k-tile
NG = 2 * D // 128   # 16 unit-groups of 128 units (64 d-values) each
N_QC = S // QC      # 4
N_KT = S // KT      # 16
W_COMP = 1.0 + 2.0 ** -9   # compensates bf16-truncation bias of the inputs

F32 = mybir.dt.float32
F32R = mybir.dt.float32r
BF16 = mybir.dt.bfloat16
U16 = mybir.dt.uint16
I32 = mybir.dt.int32


def build_body(tc, out_d, q_d, k_d, v_d, w_d):
    nc = tc.nc
    Exp = mybir.ActivationFunctionType.Exp
    AND = mybir.AluOpType.bitwise_and

    with tc.tile_pool(name="const", bufs=1) as const_pool:
        ident = const_pool.tile([128, 128], F32, tag="ident")
        make_identity(nc, ident[:])
        ones_col = const_pool.tile([128, 1], F32, tag="ones_col")
        nc.gpsimd.memset(ones_col[:], 1.0)

        # odd-partition keep-mask for the unit-interleaved layout:
        # fp32 per-partition scalar, 65535.0 on odd partitions / 0.0 on even.
        pidx = const_pool.tile([128, 1], I32, tag="pidx")
        nc.gpsimd.iota(pidx[:], pattern=[[1, 1]], base=0, channel_multiplier=1)
        podd = const_pool.tile([128, 1], I32, tag="podd")
        nc.vector.tensor_scalar(podd[:], pidx[:], 1, None, op0=AND)
        pmask_i = const_pool.tile([128, 1], I32, tag="pmask_i")
        nc.vector.tensor_scalar(pmask_i[:], podd[:], 0xFFFF, None,
                                op0=mybir.AluOpType.mult)
        andmask = const_pool.tile([128, 1], F32, tag="andmask")
        nc.vector.tensor_copy(andmask[:], pmask_i[:])

        # perm[p, 2p+1] = 1.0: as matmul lhsT it scatters 64 dense rows onto
        # odd partitions of a 128-partition PSUM tile (even rows -> 0.0).
        perm = const_pool.tile([64, 128], F32, tag="perm")
        nc.gpsimd.memset(perm[:], 1.0)
        nc.gpsimd.affine_select(
            out=perm[:],
            in_=perm[:],
            compare_op=mybir.AluOpType.is_equal,
            fill=0.0,
            base=-1,
            pattern=[[1, 128]],
            channel_multiplier=-2,
        )

        # Diagonal-block causal masks: mask[j][k_l, q_l] = 1.0 iff
        # q_l >= k_l + 128*j, else 0.0.
        masks = []
        for j in range(QC // KT):
            m = const_pool.tile([128, QC], F32, tag=f"mask{j}", name=f"mask{j}")
            nc.gpsimd.memset(m[:], 1.0)
            nc.gpsimd.affine_select(
                out=m[:],
                in_=m[:],
                compare_op=mybir.AluOpType.is_ge,
                fill=0.0,
                base=-(KT * j),
                pattern=[[1, QC]],
                channel_multiplier=-1,
            )
            masks.append(m)

        with (
            tc.tile_pool(name="xt", bufs=6) as xt_pool,
            tc.tile_pool(name="ksb", bufs=1) as k_sb_pool,
            tc.tile_pool(name="qsb", bufs=1) as q_sb_pool,
            tc.tile_pool(name="vsb", bufs=2) as v_sb_pool,
            tc.tile_pool(name="vext", bufs=1) as vext_pool,
            tc.tile_pool(name="wtmp", bufs=2) as wtmp_pool,
            tc.tile_pool(name="pacc", bufs=2, space="PSUM") as psum_acc,
            tc.tile_pool(name="pscore", bufs=2, space="PSUM") as psum_s,
            tc.tile_pool(name="pu", bufs=1, space="PSUM") as psum_u,
            tc.tile_pool(name="expp", bufs=4) as exp_pool,
            tc.tile_pool(name="usb", bufs=2) as usb_pool,
            tc.tile_pool(name="outsb", bufs=2) as out_pool,
            tc.tile_pool(name="recip", bufs=4) as recip_pool,
        ):
            # ---- input DMAs: q0..q3 first, then (k_c, v_c) pairs. Issue
            # the q transposes before anything else so the SP ring starts
            # the xbar stream at t~0.
            def issue_dma(xd, c):
                xbf = xd.ap().bitcast(BF16)  # [S, 2D] units
                xt = xt_pool.tile([128, NG, QC], BF16, tag="xt", name="xt")
                nc.sync.dma_start(
                    out=xt[:],
                    in_=xbf[c * QC:(c + 1) * QC, :],
                    transpose=True,
                )
                return xt

            q_xt = [issue_dma(q_d, c) for c in range(N_QC)]

            # ---- weights (ACT ring + a few tiny PE matmuls), overlapped
            # with the q transposes.
            w_all = []
            for wi in range(3):
                wdense = wtmp_pool.tile([64, NG, HS], F32, tag="wd", name="wd")
                nc.gpsimd.dma_start(
                    out=wdense[:],
                    in_=w_d[wi].ap().rearrange("(g j) h -> j g h", j=64),
                )
                wps = psum_s.tile([128, NG * HS // 2], F32, tag="st",
                                  name="wps")
                wps2 = psum_s.tile([128, NG * HS // 2], F32, tag="st",
                                   name="wps2")
                wflat = wdense[:].rearrange("j g h -> j (g h)")
                half = NG * HS // 2
                nc.tensor.matmul(wps[:], lhsT=perm[:], rhs=wflat[:, 0:half])
                nc.tensor.matmul(wps2[:], lhsT=perm[:],
                                 rhs=wflat[:, half:2 * half])
                wa = const_pool.tile([128, NG, HS], BF16, tag=f"w{wi}",
                                     name=f"w{wi}")
                waf = wa[:].rearrange("p g h -> p (g h)")
                nc.scalar.mul(waf[:, 0:half], wps[:], W_COMP)
                nc.scalar.mul(waf[:, half:2 * half], wps2[:], W_COMP)
                w_all.append(wa)

            def project(xt, wi, dst):
                """Sanitize even partitions of xt, project through
                w_all[wi] into dst ([64, QC] slice)."""
                flat = xt[:].rearrange("p g s -> p (g s)").bitcast(U16)
                nc.vector.tensor_scalar(flat, flat, andmask[:], None,
                                        op0=mybir.AluOpType.min)
                acc = psum_acc.tile([HS, QC], F32, tag="acc", name="acc")
                for g in range(NG):
                    nc.tensor.matmul(
                        acc[:],
                        lhsT=w_all[wi][:, g, :],
                        rhs=xt[:, g, :],
                        start=(g == 0),
                        stop=(g == NG - 1),
                    )
                nc.scalar.copy(dst, acc[:])

            # Project q3 FIRST: it depends on the last q transpose, so the
            # PE queue head consolidates the engine's startup idle into one
            # block, after which q0..q2 behind it are data-ready and the
            # matmul stream runs dense (p-state ramps once).
            qT = q_sb_pool.tile([HS, N_QC, QC], F32R, tag="qT", name="qT")
            project(q_xt[3], 0, qT[:, 3, :])
            for qc in range(N_QC - 1):
                project(q_xt[qc], 0, qT[:, qc, :])

            kT = k_sb_pool.tile([HS, S], F32R, tag="kT", name="kT")
            v_ext = [None] * N_KT
            u = [None] * N_QC
            n_done = [0] * N_QC

            for c in range(N_QC):
                k_xt = issue_dma(k_d, c)
                v_xt = issue_dma(v_d, c)
                project(k_xt, 1, kT[:, c * QC:(c + 1) * QC])
                vsb = v_sb_pool.tile([HS, QC], F32, tag="vsb", name="vsb")
                project(v_xt, 2, vsb[:])
                # v_ext[kt] = [v_rows | 1] : [128, HS+1] per k-tile.
                for j in range(QC // KT):
                    kt = c * (QC // KT) + j
                    pt = psum_s.tile([KT, QC], F32, tag="st", name="vtr")
                    nc.tensor.transpose(
                        pt[:, 0:HS],
                        vsb[:, j * KT:(j + 1) * KT],
                        ident[0:HS, 0:HS],
                    )
                    vx = vext_pool.tile([KT, HS + 1], F32R, tag=f"vext{kt}",
                                        name=f"vext{kt}")
                    nc.scalar.copy(vx[:, 0:HS], pt[:, 0:HS])
                    nc.scalar.copy(vx[:, HS:HS + 1], ones_col[:])
                    v_ext[kt] = vx

                # Score/PV blocks (qc, c) for every q chunk that attends to
                # this kv chunk; u[qc] accumulates across c in its own PSUM
                # bank and closes at c == qc.
                for qc in range(c, N_QC):
                    if u[qc] is None:
                        u[qc] = psum_u.tile([HS + 1, QC], F32, tag=f"u{qc}",
                                            name=f"u{qc}")
                    n_kt_total = (qc + 1) * (QC // KT)
                    for j in range(QC // KT):
                        kt = c * (QC // KT) + j
                        st = psum_s.tile([KT, QC], F32, tag="st", name="st")
                        nc.tensor.matmul(
                            st[:],
                            lhsT=kT[:, kt * KT:(kt + 1) * KT],
                            rhs=qT[:, qc, :],
                        )
                        et = exp_pool.tile([KT, QC], F32R, tag="et", name="et")
                        nc.scalar.activation(et[:], st[:], Exp,
                                             scale=float(HS) ** -0.5)
                        if qc == c:  # diagonal block: zero invalid region
                            nc.vector.tensor_mul(et[:], et[:], masks[j][:])
                        nc.tensor.matmul(
                            u[qc][:],
                            lhsT=v_ext[kt][:],
                            rhs=et[:],
                            start=(n_done[qc] == 0),
                            stop=(n_done[qc] == n_kt_total - 1),
                        )
                        n_done[qc] += 1

                # u[c] is complete: transpose back, normalize, store on the
                # ACT ring.
                usb = usb_pool.tile([HS + 1, QC], F32, tag="usb", name="usb")
                nc.scalar.copy(usb[:], u[c][:])
                osb = out_pool.tile([128, (QC // 128) * HS], F32,
                                    tag="osb", name="osb")
                for t in range(QC // 128):
                    po = psum_s.tile([128, QC], F32, tag="st", name="po")
                    nc.tensor.transpose(
                        po[:, 0:HS + 1],
                        usb[:, t * 128:(t + 1) * 128],
                        ident[0:HS + 1, 0:HS + 1],
                    )
                    rc = recip_pool.tile([128, 1], F32, tag="rc", name="rc")
                    nc.vector.reciprocal(rc[:], po[:, HS:HS + 1])
                    nc.vector.tensor_scalar_mul(
                        osb[:, t * HS:(t + 1) * HS], po[:, 0:HS], rc[:]
                    )
                dst = (
                    out_d.ap()[c * QC:(c + 1) * QC, :]
                    .rearrange("(t p) h -> p t h", p=128)
                )
                # SWDGE (Pool ring): keeps output stores off the HWDGE
                # queue semaphores, whose recycling barriers would otherwise
                # make later input transposes wait on earlier stores.
                nc.gpsimd.dma_start(
                    out=dst,
                    in_=osb[:].rearrange("p (t h) -> p t h", t=QC // 128),
                )


_NC_CACHE = {}


def build_nc(debug=False, reps=1):
    key = ("nc", debug, reps)
    if key in _NC_CACHE:
        return _NC_CACHE[key]
    nc = bacc.Bacc(
        "TRN2",
        target_bir_lowering=False,
        debug=debug,
        num_devices=N_CORES,
    )
    q_d = nc.dram_tensor("query", [S, D], F32, kind="ExternalInput")
    k_d = nc.dram_tensor("key", [S, D], F32, kind="ExternalInput")
    v_d = nc.dram_tensor("value", [S, D], F32, kind="ExternalInput")
    wq_d = nc.dram_tensor("Wq", [D, HS], F32, kind="ExternalInput")
    wk_d = nc.dram_tensor("Wk", [D, HS], F32, kind="ExternalInput")
    wv_d = nc.dram_tensor("Wv", [D, HS], F32, kind="ExternalInput")
    out_d = nc.dram_tensor("out", [S, HS], F32, kind="ExternalOutput")

    with tile.TileContext(nc) as tc:
        for _ in range(reps):
            build_body(tc, out_d, q_d, k_d, v_d, [wq_d, wk_d, wv_d])
    nc.compile()
    _NC_CACHE[key] = nc
    return nc


def make_in_maps(query, key, value, Wq, Wk, Wv):
    query = np.ascontiguousarray(query, dtype=np.float32)
    key = np.ascontiguousarray(key, dtype=np.float32)
    value = np.ascontiguousarray(value, dtype=np.float32)
    Wq = np.ascontiguousarray(Wq, dtype=np.float32)
    Wk = np.ascontiguousarray(Wk, dtype=np.float32)
    Wv = np.ascontiguousarray(Wv, dtype=np.float32)
    return [
        {
            "query": query[b],
            "key": key[b],
            "value": value[b],
            "Wq": Wq,
            "Wk": Wk,
            "Wv": Wv,
        }
        for b in range(N_CORES)
    ]


def kernel(query, key, value, Wq, Wk, Wv, trace=False):
    from concourse.bass_utils import run_bass_kernel_spmd

    nc = build_nc()
    in_maps = make_in_maps(query, key, value, Wq, Wk, Wv)
    res = run_bass_kernel_spmd(nc, in_maps, core_ids=list(range(N_CORES)), trace=trace)
    out = np.stack([res.results[b]["out"] for b in range(N_CORES)], axis=0)
    if trace:
        kernel.last_results = res
    return out
